# revision 1
# baseline (speedup 1.0000x reference)
"""AdderNet BasicBlock (adder conv ×2 + BN + SE + residual) on 8 TRN2 cores.

Data-parallel over batch N=16 -> 2 images per core. Inside each core:
  - adder2d: out[co,p] = -sum_{ci,off} |x[ci, p+off] - w[co,ci,off]|
    * |x - w| tiles [128ci, 2048] computed on ScalarE (Abs(w - x) via
      per-partition bias, fp16 in / fp16 out) and VectorE (fp16
      tensor_scalar subtract at 4x mode + sign-bit clear via bitvec AND),
      co-interleaved so TensorE consumes from both engines.
    * partition reduction + (co,off) accumulation on TensorE via one-hot
      column matmuls into PSUM [128co, 2048pos].
  - BN(+ReLU) folded to per-channel scale/bias, one ACT instr from PSUM.
  - SE gate: DVE reduce -> 2 small matmuls + Relu/Sigmoid.
  - residual: (bn2*gate) + x via scalar_tensor_tensor, then ReLU.

fp16 is used for the DVE |x-w| tiles and fp8e4 (paired into DoubleRow
matmuls) for the ScalarE tiles; reductions/PSUM/BN/SE all stay fp32.
End-to-end error vs the fp32 reference ~1.2e-3 (8 cores, ~1.79 ms HW).
"""

import numpy as np
from itertools import product

import concourse.bacc as bacc
import concourse.bass as bass
import concourse.mybir as mybir
import concourse.tile as tile
from concourse.bass_utils import run_bass_kernel_spmd

F32 = mybir.dt.float32
F16 = mybir.dt.float16
U16 = mybir.dt.uint16
AF = mybir.ActivationFunctionType
ALU = mybir.AluOpType

N_CORES = 8
N, C, H, W = 16, 128, 32, 32
NPC = N // N_CORES          # images per core
HP, WP = H + 2, W + 2       # padded
POS = H * W                 # 1024
FREE = NPC * POS            # 2048 free elems per conv instruction
PADF = NPC * HP * WP        # 2312 flat padded size
KK = 9                      # 3x3
EPS = 1e-5

# co -> engine assignment: True = ScalarE(ACT), False = VectorE(DVE).
N_ACT_COS = 50              # of 128, evenly interleaved
DVE_ABS_MODE = "bitvec"     # "bitvec" | "stt"
ACT_FP8 = True              # ACT-cos emit fp8e4 tiles, paired DoubleRow mms
F8 = mybir.dt.float8e4
PM = mybir.MatmulPerfMode


def _use_act(co: int) -> bool:
    return (co * N_ACT_COS) // 128 != ((co + 1) * N_ACT_COS) // 128


_DVE_COS = [c for c in range(C) if not ((c * N_ACT_COS) // 128 != ((c + 1) * N_ACT_COS) // 128)]
MIX_COS = frozenset(_DVE_COS[7::16])


def _src_view(padA, padB, dh, dw):
    off = dh * WP + dw
    if off % 2 == 0:
        return padA[:, :, dh:dh + H, dw:dw + W]
    return padB[:].rearrange(
        "p (i h w) -> p i h w", i=NPC, h=HP, w=WP)[
        :, :, dh:dh + H, dw - 1:dw - 1 + W]


OFFS = list(product(range(3), range(3)))


def _conv_layer(nc, padA, padB, wT, psum, pools, Z16, Z8):
    """One adder conv.

    padA/padB: [128, NPC, HP, WP] fp16, B shifted left by one element so
    odd window offsets stay 4-byte aligned (DVE 4x mode).
    -> psum [128co, FREE] accumulates sum over (ci, off) of |x - w|.
    """
    abs_pool, d_pool, pair_pool, s8_pool, s16_pool = pools
    for co in range(C):
        if _use_act(co) and ACT_FP8:
            # 4 offset-pairs as fp8 DoubleRow + 1 single fp8 matmul set
            for pi in range(4):
                pair = pair_pool.tile([128, 2, NPC, H, W], F8, tag="pair")
                for k in range(2):
                    dh, dw = OFFS[2 * pi + k]
                    o = 2 * pi + k
                    col = wT[:, co * KK + o: co * KK + o + 1]
                    nc.scalar.activation(
                        pair[:, k], _src_view(padA, padB, dh, dw),
                        AF.Abs, bias=col, scale=-1.0)
                pf = pair[:].rearrange("p t i h w -> p t (i h w)")
                lhsT8 = Z8[:, :, 128 - co:256 - co]
                for cc in range(FREE // 512):
                    nc.tensor.matmul(
                        psum[:, 512 * cc:512 * (cc + 1)],
                        lhsT8,
                        pf[:, :, 512 * cc:512 * (cc + 1)],
                        start=(co == 0 and pi == 0),
                        stop=False,
                        perf_mode=PM.DoubleRow,
                    )
            # leftover 9th offset
            dh, dw = OFFS[8]
            col = wT[:, co * KK + 8: co * KK + 9]
            t8 = s8_pool.tile([128, NPC, H, W], F8, tag="s8")
            nc.scalar.activation(t8[:], _src_view(padA, padB, dh, dw),
                                 AF.Abs, bias=col, scale=-1.0)
            t8f = t8[:].rearrange("p i h w -> p (i h w)")
            lhsT8s = Z8[:, 0, 128 - co:256 - co]
            for cc in range(FREE // 512):
                nc.tensor.matmul(
                    psum[:, 512 * cc:512 * (cc + 1)],
                    lhsT8s,
                    t8f[:, 512 * cc:512 * (cc + 1)],
                    start=False, stop=(co == C - 1))
            continue
        if _use_act(co):
            # non-fp8 ACT path (ACT_FP8 False)
            lhsT = Z16[:, 128 - co:256 - co]
            for o, (dh, dw) in enumerate(OFFS):
                col = wT[:, co * KK + o: co * KK + o + 1]
                t = abs_pool.tile([128, NPC, H, W], F16, tag="abs")
                nc.scalar.activation(t[:], _src_view(padA, padB, dh, dw),
                                     AF.Abs, bias=col, scale=-1.0)
                tf = t[:].rearrange("p i h w -> p (i h w)")
                for cc in range(FREE // 512):
                    nc.tensor.matmul(
                        psum[:, 512 * cc:512 * (cc + 1)], lhsT,
                        tf[:, 512 * cc:512 * (cc + 1)],
                        start=(co == 0 and o == 0), stop=False)
            continue
        # DVE path: subtract pairs of offsets, one sign-clear AND per pair
        lhsT = Z16[:, 128 - co:256 - co]
        npairs = 4 if co in MIX_COS else 5
        for pi in range(npairs):
            ks = (0, 1) if pi < 4 else (0,)
            d2 = d_pool.tile([128, 2, NPC * H * W], F16, tag="d")
            for k in ks:
                o = 2 * pi + k
                dh, dw = OFFS[o]
                col = wT[:, co * KK + o: co * KK + o + 1]
                nc.vector.tensor_scalar(
                    d2[:, k], _src_view(padA, padB, dh, dw), col, None,
                    op0=ALU.subtract, op1=ALU.bypass)
            t2 = abs_pool.tile([128, 2, NPC * H * W], F16, tag="abs")
            nwords = len(ks) * NPC * H * W
            nc.vector.tensor_scalar(
                t2[:].rearrange("p t f -> p (t f)")[:, :nwords].bitcast(U16),
                d2[:].rearrange("p t f -> p (t f)")[:, :nwords].bitcast(U16),
                0x7FFF, None, op0=ALU.bitwise_and, op1=ALU.bypass)
            for k in ks:
                o = 2 * pi + k
                for cc in range(FREE // 512):
                    nc.tensor.matmul(
                        psum[:, 512 * cc:512 * (cc + 1)], lhsT,
                        t2[:, k, 512 * cc:512 * (cc + 1)],
                        start=(co == 0 and o == 0),
                        stop=(co == C - 1 and o == KK - 1),
                    )
        if co in MIX_COS:
            dh, dw = OFFS[8]
            col = wT[:, co * KK + 8: co * KK + 9]
            t8 = s8_pool.tile([128, NPC, H, W], F8, tag="s8")
            nc.scalar.activation(t8[:], _src_view(padA, padB, dh, dw),
                                 AF.Abs, bias=col, scale=-1.0)
            t8f = t8[:].rearrange("p i h w -> p (i h w)")
            lhsT8s = Z8[:, 0, 128 - co:256 - co]
            for cc in range(FREE // 512):
                nc.tensor.matmul(
                    psum[:, 512 * cc:512 * (cc + 1)], lhsT8s,
                    t8f[:, 512 * cc:512 * (cc + 1)],
                    start=False, stop=False)


def _build_nc():
    nc = bacc.Bacc("TRN2", target_bir_lowering=False, debug=False,
                   num_devices=N_CORES)

    x_d = nc.dram_tensor("x", [NPC, C, H, W], F32, kind="ExternalInput")
    wT1_d = nc.dram_tensor("wT1", [C, C * KK], F32, kind="ExternalInput")
    wT2_d = nc.dram_tensor("wT2", [C, C * KK], F32, kind="ExternalInput")
    bnc_d = nc.dram_tensor("bnc", [C, 4], F32, kind="ExternalInput")
    fc1T_d = nc.dram_tensor("fc1T", [C, 8], F32, kind="ExternalInput")
    fc1b_d = nc.dram_tensor("fc1b", [8, 1], F32, kind="ExternalInput")
    fc2T_d = nc.dram_tensor("fc2T", [8, C], F32, kind="ExternalInput")
    fc2b_d = nc.dram_tensor("fc2b", [C, 1], F32, kind="ExternalInput")
    out_d = nc.dram_tensor("out", [NPC, C, H, W], F32, kind="ExternalOutput")

    xa, outa = x_d.ap(), out_d.ap()

    with tile.TileContext(nc) as tc:
        with (
            tc.tile_pool(name="const", bufs=1) as cpool,
            tc.tile_pool(name="pad", bufs=1) as padpool,
            tc.tile_pool(name="absp", bufs=5) as abs_pool,
            tc.tile_pool(name="dp", bufs=3) as d_pool,
            tc.tile_pool(name="s16p", bufs=1) as s16_pool,
            tc.tile_pool(name="pairp", bufs=5) as pair_pool,
            tc.tile_pool(name="s8p", bufs=6) as s8_pool,
            tc.tile_pool(name="misc", bufs=1) as mpool,
            tc.tile_pool(name="psum", bufs=1, space=bass.MemorySpace.PSUM) as pp,
            tc.tile_pool(name="psum_se", bufs=2, space=bass.MemorySpace.PSUM) as pps,
        ):
            # constants
            Z16 = cpool.tile([128, 256], F16, tag="Z16")   # one-hot bank
            nc.vector.memset(Z16[:], 0.0)
            nc.vector.memset(Z16[:, 128:129], 1.0)
            sgw = cpool.tile([128, 2], F32, tag="sgw")
            nc.vector.memset(sgw[:], 0.0)
            nc.scalar.activation(sgw[:, 1:2], sgw[:, 0:1], AF.Sigmoid)
            Z8 = cpool.tile([128, 2, 256], F8, tag="Z8")
            nc.vector.memset(Z8[:], 0.0)
            nc.vector.memset(Z8[:, :, 128:129], 1.0)
            pools = (abs_pool, d_pool, pair_pool, s8_pool, s16_pool)
            wT1 = cpool.tile([C, C * KK], F32, tag="wT1")
            nc.sync.dma_start(wT1[:], wT1_d.ap())
            wT2 = cpool.tile([C, C * KK], F32, tag="wT2")
            nc.sync.dma_start(wT2[:], wT2_d.ap())
            bnc = cpool.tile([C, 4], F32, tag="bnc")
            nc.sync.dma_start(bnc[:], bnc_d.ap())
            fc1T = cpool.tile([C, 8], F32, tag="fc1T")
            nc.sync.dma_start(fc1T[:], fc1T_d.ap())
            fc1b = cpool.tile([8, 1], F32, tag="fc1b")
            nc.sync.dma_start(fc1b[:], fc1b_d.ap())
            fc2T = cpool.tile([8, C], F32, tag="fc2T")
            nc.sync.dma_start(fc2T[:], fc2T_d.ap())
            fc2b = cpool.tile([C, 1], F32, tag="fc2b")
            nc.sync.dma_start(fc2b[:], fc2b_d.ap())

            # padded fp32 input (kept for the residual add)
            xpad = padpool.tile([128, NPC, HP, WP], F32, tag="xpad")
            nc.vector.memset(xpad[:], 0.0)
            for i in range(NPC):
                nc.sync.dma_start(xpad[:, i, 1:1 + H, 1:1 + W], xa[i])
            # fp16 A/B copies for the conv reads
            xA = padpool.tile([128, NPC, HP, WP], F16, tag="xA")
            xB = padpool.tile([128, PADF], F16, tag="xB")
            xpf = xpad[:].rearrange("p i h w -> p (i h w)")
            xAf = xA[:].rearrange("p i h w -> p (i h w)")
            nc.vector.tensor_copy(xAf, xpf)
            nc.vector.memset(xB[:, PADF - 1:PADF], 0.0)
            nc.vector.tensor_copy(xB[:, 0:PADF - 1], xpf[:, 1:PADF])

            # ---- conv1 + BN1 + ReLU -> out1 fp16 A/B ----
            with nc.named_scope("conv1"):
                psum1 = pp.tile([128, FREE], F32, tag="big")
                _conv_layer(nc, xA, xB, wT1, psum1, pools, Z16, Z8)
                o1A = padpool.tile([128, NPC, HP, WP], F16, tag="o1A")
                o1B = padpool.tile([128, PADF], F16, tag="o1B")
                nc.vector.memset(o1A[:], 0.0)
                # bn1: relu(-a1 * s + b1); psum holds s = sum|x-w| >= 0
                nc.scalar.activation(
                    o1A[:, :, 1:1 + H, 1:1 + W],
                    psum1[:].rearrange("p (i h w) -> p i h w", i=NPC, h=H, w=W),
                    AF.Relu, bias=bnc[:, 1:2], scale=bnc[:, 0:1])
                o1Af = o1A[:].rearrange("p i h w -> p (i h w)")
                nc.vector.memset(o1B[:, PADF - 1:PADF], 0.0)
                nc.vector.tensor_copy(
                    o1B[:, 0:PADF - 1].bitcast(U16), o1Af[:, 1:PADF].bitcast(U16))

            # ---- conv2 + BN2 -> bn2out fp32 ----
            with nc.named_scope("conv2"):
                psum2 = pp.tile([128, FREE], F32, tag="big")
                _conv_layer(nc, o1A, o1B, wT2, psum2, pools, Z16, Z8)
                bn2out = mpool.tile([128, FREE], F32, tag="bn2out")
                nc.scalar.activation(
                    bn2out[:], psum2[:], AF.Identity,
                    bias=bnc[:, 3:4], scale=bnc[:, 2:3])

            # ---- SE gate ----
            with nc.named_scope("se"):
                pooled = mpool.tile([128, NPC], F32, tag="pooled")
                for i in range(NPC):
                    nc.vector.reduce_sum(
                        pooled[:, i:i + 1], bn2out[:, POS * i:POS * (i + 1)],
                        axis=mybir.AxisListType.X)
                ps_se1 = pps.tile([8, NPC], F32, tag="se")
                # fc1T pre-scaled by 1/POS (mean folded in)
                nc.tensor.matmul(ps_se1[:], fc1T[:], pooled[:],
                                 start=True, stop=True)
                s2 = mpool.tile([8, NPC], F32, tag="s2")
                nc.scalar.activation(s2[:], ps_se1[:], AF.Relu,
                                     bias=fc1b[:, 0:1])
                ps_se2 = pps.tile([128, NPC], F32, tag="se")
                nc.tensor.matmul(ps_se2[:], fc2T[:], s2[:],
                                 start=True, stop=True)
                gate = mpool.tile([128, NPC], F32, tag="gate")
                nc.scalar.activation(gate[:], ps_se2[:], AF.Sigmoid,
                                     bias=fc2b[:, 0:1])

                # ---- residual + final relu + store ----
                outsb = mpool.tile([128, FREE], F32, tag="outsb")
                bn4 = bn2out[:].rearrange("p (i h w) -> p i h w",
                                          i=NPC, h=H, w=W)
                o4 = outsb[:].rearrange("p (i h w) -> p i h w",
                                        i=NPC, h=H, w=W)
                for i in range(NPC):
                    t2 = mpool.tile([128, H, W], F32, tag="t2")
                    nc.vector.scalar_tensor_tensor(
                        t2[:], bn4[:, i], gate[:, i:i + 1],
                        xpad[:, i, 1:1 + H, 1:1 + W],
                        op0=ALU.mult, op1=ALU.add)
                    nc.scalar.activation(o4[:, i], t2[:], AF.Relu)
                    nc.sync.dma_start(outa[i], o4[:, i])

    nc.compile()
    return nc


_NC_CACHE = None


def _get_nc():
    global _NC_CACHE
    if _NC_CACHE is None:
        _NC_CACHE = _build_nc()
    return _NC_CACHE


def _host_prep(inputs):
    f = np.float32
    w1 = np.ascontiguousarray(inputs["w1"], dtype=f)
    w2 = np.ascontiguousarray(inputs["w2"], dtype=f)
    # [co, ci, kh, kw] -> [ci, co*9 + off]
    wT1 = np.ascontiguousarray(w1.transpose(1, 0, 2, 3).reshape(C, C * KK))
    wT2 = np.ascontiguousarray(w2.transpose(1, 0, 2, 3).reshape(C, C * KK))

    def bn_fold(g, b, m, v):
        g, b, m, v = (np.asarray(t, np.float64) for t in (g, b, m, v))
        a = g / np.sqrt(v + EPS)
        return (-a).astype(f), (b - m * a).astype(f)

    s1, b1 = bn_fold(inputs["bn1_gamma"], inputs["bn1_beta"],
                     inputs["bn1_mean"], inputs["bn1_var"])
    s2, b2 = bn_fold(inputs["bn2_gamma"], inputs["bn2_beta"],
                     inputs["bn2_mean"], inputs["bn2_var"])
    bnc = np.ascontiguousarray(np.stack([s1, b1, s2, b2], axis=1))

    fc1T = np.ascontiguousarray(inputs["fc1_w"].astype(f).T / np.float32(POS))
    fc1b = np.ascontiguousarray(inputs["fc1_b"].astype(f).reshape(8, 1))
    fc2T = np.ascontiguousarray(inputs["fc2_w"].astype(f).T)
    fc2b = np.ascontiguousarray(inputs["fc2_b"].astype(f).reshape(C, 1))
    return dict(wT1=wT1, wT2=wT2, bnc=bnc, fc1T=fc1T, fc1b=fc1b,
                fc2T=fc2T, fc2b=fc2b)


def run(inputs, trace=False, tmpdir=None):
    nc = _get_nc()
    shared = _host_prep(inputs)
    x = np.ascontiguousarray(inputs["x"], dtype=np.float32)
    in_maps = []
    for i in range(N_CORES):
        m = dict(shared)
        m["x"] = np.ascontiguousarray(x[i * NPC:(i + 1) * NPC])
        in_maps.append(m)
    res = run_bass_kernel_spmd(nc, in_maps, core_ids=list(range(N_CORES)),
                               trace=trace, tmpdir=tmpdir)
    out = np.concatenate([res.results[i]["out"] for i in range(N_CORES)], 0)
    return out, res


def kernel(**inputs) -> np.ndarray:
    out, _ = run(inputs)
    return out



# revision 3
# speedup vs baseline: 2.0693x; 2.0693x over previous
"""AdderNet BasicBlock v2: fp8 DoubleRow threshold matmuls. 8 TRN2 cores.

Same math as v1 (threshold decomposition of the adder L1-distance + linear
tail + host bias correction) with:
  - grid planes in fp8e4 paired into DoubleRow matmuls (2 planes/instr)
  - linear-tail plane 9-offset box-summed on DVE -> 1 matmul per chunk
    with an all-ones stationary (weights folded: ones)
  - per-image PSUM tiles -> enc2/conv2/BN2/SE pipeline across images
  - SE gate computed per image (it is per-image in the reference)
  - PE warm-up matmuls at t=0 to lift the HAM clock gate to 2.4 GHz
  - pad-region memsets as thin strips on GPSIMD instead of full-tile DVE
Grid steps snapped up to fp8-exact values so every matmul operand is exact.
"""

import numpy as np
import ml_dtypes

import concourse.bacc as bacc
import concourse.bass as bass
import concourse.mybir as mybir
import concourse.tile as tile
from concourse.bass_utils import run_bass_kernel_spmd

F32 = mybir.dt.float32
F16 = mybir.dt.float16
F8 = mybir.dt.float8e4
U8 = mybir.dt.uint8
AF = mybir.ActivationFunctionType
ALU = mybir.AluOpType
PM = mybir.MatmulPerfMode

N_CORES = 8
N, C, H, W = 16, 128, 32, 32
NPC = N // N_CORES
HP, WP = H + 2, W + 2
PPI = HP * WP                    # 1156
POS = H * W
WIN = (H - 1) * WP + W           # 1086
KK = 9
EPS = 1e-5

T1 = 4                           # layer-1 grid planes (2 DR pairs)
T2P = 4                          # layer-2 positive-threshold planes (2 DR pairs)
OFFS = [dh * WP + dw for dh in range(3) for dw in range(3)]
CHUNKS = [(0, 512), (512, 512), (1024, WIN - 1024)]
N_WARM = 10
ACC = True                       # use ACT accum_out for SE pooling

# plane conventions: index -> 'sign' (ACT, +-1) or 'gt' (DVE, 0/1)
CONV1 = ("sign", "gt", "gt", "sign")
CONV2 = ("sign", "gt", "gt", "sign")


def _enc0(kind, th):
    return -np.sign(th) if kind == "sign" else float(0.0 > th)


def _plane_w(kind, w, th, delta):
    if kind == "sign":
        return -(delta / 2.0) * (2.0 * (w > th) - 1.0)
    return delta * (1.0 - 2.0 * (w > th))


def _build_nc():
    nc = bacc.Bacc("TRN2", target_bir_lowering=False, debug=False,
                   num_devices=N_CORES)

    x_d = nc.dram_tensor("x", [NPC, C, H, W], F32, kind="ExternalInput")
    A1_d = nc.dram_tensor("A1", [C, KK * T1 * C], U8, kind="ExternalInput")
    A2_d = nc.dram_tensor("A2", [C, KK * T2P * C], U8, kind="ExternalInput")
    bnc_d = nc.dram_tensor("bnc", [C, 12], F32, kind="ExternalInput")
    fc1T_d = nc.dram_tensor("fc1T", [C, 8], F32, kind="ExternalInput")
    fc1b_d = nc.dram_tensor("fc1b", [8, 1], F32, kind="ExternalInput")
    fc2T_d = nc.dram_tensor("fc2T", [8, C], F32, kind="ExternalInput")
    fc2b_d = nc.dram_tensor("fc2b", [C, 1], F32, kind="ExternalInput")
    out_d = nc.dram_tensor("out", [NPC, C, H, W], F32, kind="ExternalOutput")

    xa, outa = x_d.ap(), out_d.ap()
    # DVE enc1 thresholds come in as python floats at build time -> must be
    # runtime data instead: passed via host_prep module attribute.
    th1_dve = _build_nc.th1_dve      # two floats, layer-1 'gt' thresholds

    with tile.TileContext(nc) as tc:
        with (
            tc.tile_pool(name="const", bufs=1) as cpool,
            tc.tile_pool(name="pad", bufs=1) as padpool,
            tc.tile_pool(name="planes", bufs=1) as plpool,
            tc.tile_pool(name="misc", bufs=1) as mpool,
            tc.tile_pool(name="psum", bufs=2, space=bass.MemorySpace.PSUM) as pp,
            tc.tile_pool(name="psum_se", bufs=2, space=bass.MemorySpace.PSUM) as pps,
        ):
            # ---- warm-up: keep PE busy so the HAM clock ramps early ----
            ones = cpool.tile([C, C], F16, tag="ones")
            nc.vector.memset(ones[:], 1.0)
            scr = cpool.tile([C, 512], F16, tag="scr")
            nc.vector.memset(scr[:], 0.0)
            wps = pps.tile([C, 512], F32, tag="se")
            for _ in range(N_WARM):
                nc.tensor.matmul(wps[:], ones[:], scr[:], start=True, stop=True)

            # ---- constants ----
            A1 = cpool.tile([C, KK, 2, 2, C], U8, tag="A1")
            nc.sync.dma_start(A1[:].rearrange("p a b c d -> p (a b c d)"), A1_d.ap())
            A2 = cpool.tile([C, KK, 2, 2, C], U8, tag="A2")
            nc.sync.dma_start(A2[:].rearrange("p a b c d -> p (a b c d)"), A2_d.ap())
            bnc = cpool.tile([C, 12], F32, tag="bnc")
            nc.sync.dma_start(bnc[:], bnc_d.ap())
            fc1T = cpool.tile([C, 8], F32, tag="fc1T")
            nc.sync.dma_start(fc1T[:], fc1T_d.ap())
            fc1b = cpool.tile([8, 1], F32, tag="fc1b")
            nc.sync.dma_start(fc1b[:], fc1b_d.ap())
            fc2T = cpool.tile([8, C], F32, tag="fc2T")
            nc.sync.dma_start(fc2T[:], fc2T_d.ap())
            fc2b = cpool.tile([C, 1], F32, tag="fc2b")
            nc.sync.dma_start(fc2b[:], fc2b_d.ap())

            def A8(t, o, j):
                return t[:, o, j].bitcast(F8)

            # ---- padded input: strips zeroed, interior DMA'd ----
            xpad = padpool.tile([C, NPC, HP, WP], F32, tag="xpad")
            nc.vector.memset(xpad[:, :, 0, :], 0.0)
            nc.vector.memset(xpad[:, :, HP - 1, :], 0.0)
            nc.vector.memset(xpad[:, :, 1:1 + H, 0], 0.0)
            nc.vector.memset(xpad[:, :, 1:1 + H, WP - 1], 0.0)
            for i in range(NPC):
                nc.sync.dma_start(xpad[:, i, 1:1 + H, 1:1 + W], xa[i])
            xpf = xpad[:].rearrange("p i h w -> p (i h w)")

            # ---- layer-1 encodings ----
            E1 = plpool.tile([C, 2, 2, NPC * PPI], F8, tag="E1")
            # pair0: plane0 (ACT sign), plane1 (DVE gt)
            nc.scalar.activation(E1[:, 0, 0].bitcast(F8), xpf, AF.Sign,
                                 bias=bnc[:, 3:4])
            nc.vector.tensor_scalar(E1[:, 0, 1].bitcast(F8), xpf,
                                    th1_dve[0], None, op0=ALU.is_gt)
            nc.scalar.activation(E1[:, 1, 1].bitcast(F8), xpf, AF.Sign,
                                 bias=bnc[:, 4:5])
            nc.vector.tensor_scalar(E1[:, 1, 0].bitcast(F8), xpf,
                                    th1_dve[1], None, op0=ALU.is_gt)
            # linear tail: v = relu(|x| - c1), then 3x3 box sum on DVE
            ab = padpool.tile([C, NPC, HP, WP], F16, tag="ab")
            nc.scalar.activation(ab[:].rearrange("p i h w -> p (i h w)"),
                                 xpf, AF.Abs)
            v1t = padpool.tile([C, NPC, HP, WP], F16, tag="v1t")
            nc.scalar.activation(v1t[:].rearrange("p i h w -> p (i h w)"),
                                 ab[:].rearrange("p i h w -> p (i h w)"),
                                 AF.Relu, bias=bnc[:, 5:6])
            rs1 = padpool.tile([C, NPC, HP, WP], F16, tag="rs1")
            nc.vector.tensor_tensor(rs1[:, :, :, 0:W], v1t[:, :, :, 0:W],
                                    v1t[:, :, :, 1:1 + W], op=ALU.add)
            nc.vector.tensor_tensor(rs1[:, :, :, 0:W], rs1[:, :, :, 0:W],
                                    v1t[:, :, :, 2:2 + W], op=ALU.add)
            vs1 = padpool.tile([C, NPC, HP, WP], F16, tag="vs1")
            nc.vector.tensor_tensor(vs1[:, :, 0:H, :], rs1[:, :, 0:H, :],
                                    rs1[:, :, 1:1 + H, :], op=ALU.add)
            nc.vector.tensor_tensor(vs1[:, :, 0:H, :], vs1[:, :, 0:H, :],
                                    rs1[:, :, 2:2 + H, :], op=ALU.add)
            vs1f = vs1[:].rearrange("p i h w -> p (i h w)")

            # ---- layer-2 plane pad strips (GPSIMD) ----
            E2 = plpool.tile([C, 2, 2, NPC, HP, WP], F8, tag="E2")
            PADV = {(0, 0): -1.0, (0, 1): 0.0, (1, 0): 0.0, (1, 1): -1.0}
            for (j, k), pv in PADV.items():
                pl = E2[:, j, k].bitcast(F8)
                nc.gpsimd.memset(pl[:, :, 0, :], pv)
                nc.gpsimd.memset(pl[:, :, HP - 1, :], pv)
                nc.gpsimd.memset(pl[:, :, 1:1 + H, 0], pv)
                nc.gpsimd.memset(pl[:, :, 1:1 + H, WP - 1], pv)
            v2t = padpool.tile([C, NPC, HP, WP], F16, tag="v2t")
            nc.gpsimd.memset(v2t[:, :, 0, :], 0.0)
            nc.gpsimd.memset(v2t[:, :, HP - 1, :], 0.0)
            nc.gpsimd.memset(v2t[:, :, 1:1 + H, 0], 0.0)
            nc.gpsimd.memset(v2t[:, :, 1:1 + H, WP - 1], 0.0)

            def conv(eg, vsf, At, psums, scope):
                with nc.named_scope(scope):
                    for i in range(NPC):
                        for base, Lc in CHUNKS:
                            first = True
                            for j in range(2):
                                for o in range(KK):
                                    w0 = i * PPI + OFFS[o] + base
                                    nc.tensor.matmul(
                                        psums[i][:, base:base + Lc],
                                        A8(At, o, j),
                                        eg[:, j, :, w0:w0 + Lc].bitcast(F8),
                                        start=first, stop=False,
                                        perf_mode=PM.DoubleRow)
                                    first = False
                            nc.tensor.matmul(
                                psums[i][:, base:base + Lc], ones[:],
                                vsf[:, i * PPI + base:i * PPI + base + Lc],
                                start=False, stop=True)

            # E1 flat view with pair dim: [C, 2, 2, F]
            e1v = E1[:]
            psum1 = [pp.tile([C, 3 * 512], F32, tag="conv", name=f"psum1_{i_}")
                     for i_ in range(NPC)]
            conv(e1v, vs1f, A1, psum1, "conv1")

            # ---- enc2 + box2 per image (from psum1), then conv2 ----
            rs2 = padpool.tile([C, NPC, HP, WP], F16, tag="rs2")
            vs2 = padpool.tile([C, NPC, HP, WP], F16, tag="vs2")
            vs2f = vs2[:].rearrange("p i h w -> p (i h w)")
            psum2 = []

            def enc2_img(i):
                with nc.named_scope("enc2"):
                    pv = psum1[i][:, 0:H * WP].rearrange(
                        "p (h w) -> p h w", h=H, w=WP)[:, :, 0:W]
                    nc.scalar.activation(
                        E2[:, 0, 0, i, 1:1 + H, 1:1 + W].bitcast(F8), pv,
                        AF.Sign, bias=bnc[:, 6:7], scale=bnc[:, 0:1])
                    nc.vector.tensor_scalar(
                        E2[:, 0, 1, i, 1:1 + H, 1:1 + W].bitcast(F8), pv,
                        bnc[:, 0:1], bnc[:, 9:10], op0=ALU.mult, op1=ALU.is_gt)
                    nc.vector.tensor_scalar(
                        E2[:, 1, 0, i, 1:1 + H, 1:1 + W].bitcast(F8), pv,
                        bnc[:, 0:1], bnc[:, 10:11], op0=ALU.mult, op1=ALU.is_gt)
                    nc.scalar.activation(
                        E2[:, 1, 1, i, 1:1 + H, 1:1 + W].bitcast(F8), pv,
                        AF.Sign, bias=bnc[:, 7:8], scale=bnc[:, 0:1])
                    nc.scalar.activation(
                        v2t[:, i, 1:1 + H, 1:1 + W], pv,
                        AF.Relu, bias=bnc[:, 8:9], scale=bnc[:, 0:1])
                    nc.vector.tensor_tensor(rs2[:, i, :, 0:W], v2t[:, i, :, 0:W],
                                            v2t[:, i, :, 1:1 + W], op=ALU.add)
                    nc.vector.tensor_tensor(rs2[:, i, :, 0:W], rs2[:, i, :, 0:W],
                                            v2t[:, i, :, 2:2 + W], op=ALU.add)
                    nc.vector.tensor_tensor(vs2[:, i, 0:H, :], rs2[:, i, 0:H, :],
                                            rs2[:, i, 1:1 + H, :], op=ALU.add)
                    nc.vector.tensor_tensor(vs2[:, i, 0:H, :], vs2[:, i, 0:H, :],
                                            rs2[:, i, 2:2 + H, :], op=ALU.add)

            def conv2_img(i):
                ps = pp.tile([C, 3 * 512], F32, tag="conv", name=f"psum2_{i}")
                psum2.append(ps)
                with nc.named_scope("conv2"):
                    for base, Lc in CHUNKS:
                        first = True
                        for j in range(2):
                            for o in range(KK):
                                w0 = i * PPI + OFFS[o] + base
                                nc.tensor.matmul(
                                    ps[:, base:base + Lc],
                                    A8(A2, o, j),
                                    E2[:, j, :, :].rearrange(
                                        "p k i h w -> p k (i h w)")[:, :, w0:w0 + Lc].bitcast(F8),
                                    start=first, stop=False,
                                    perf_mode=PM.DoubleRow)
                                first = False
                        nc.tensor.matmul(
                            ps[:, base:base + Lc], ones[:],
                            vs2f[:, i * PPI + base:i * PPI + base + Lc],
                            start=False, stop=True)

            bn2sb = mpool.tile([C, NPC, H, W], F32, tag="bn2sb")
            pooled = mpool.tile([C, NPC], F32, tag="pooled")
            outsb = mpool.tile([C, NPC, H, W], F32, tag="outsb")

            def tail_img(i):
                with nc.named_scope("tail"):
                    pv = psum2[i][:, 0:H * WP].rearrange(
                        "p (h w) -> p h w", h=H, w=WP)[:, :, 0:W]
                    if ACC:
                        nc.scalar.activation(bn2sb[:, i], pv, AF.Identity,
                                             bias=bnc[:, 2:3], scale=bnc[:, 1:2],
                                             accum_out=pooled[:, i:i + 1])
                    else:
                        nc.scalar.activation(bn2sb[:, i], pv, AF.Identity,
                                             bias=bnc[:, 2:3], scale=bnc[:, 1:2])
                        nc.vector.reduce_sum(
                            pooled[:, i:i + 1],
                            bn2sb[:, i].rearrange("p h w -> p (h w)"),
                            axis=mybir.AxisListType.X)
                    ps1 = pps.tile([8, 1], F32, tag="se", name=f"ps1_{i}")
                    nc.tensor.matmul(ps1[:], fc1T[:], pooled[:, i:i + 1],
                                     start=True, stop=True)
                    s2 = mpool.tile([8, NPC], F32, tag="s2")
                    nc.scalar.activation(s2[:, i:i + 1], ps1[:], AF.Relu,
                                         bias=fc1b[:, 0:1])
                    ps2_ = pps.tile([C, 1], F32, tag="se", name=f"ps2_{i}")
                    nc.tensor.matmul(ps2_[:], fc2T[:], s2[:, i:i + 1],
                                     start=True, stop=True)
                    gate = mpool.tile([C, NPC], F32, tag="gate")
                    nc.scalar.activation(gate[:, i:i + 1], ps2_[:], AF.Sigmoid,
                                         bias=fc2b[:, 0:1])
                    t2 = mpool.tile([C, H, W], F32, tag="t2")
                    nc.vector.scalar_tensor_tensor(
                        t2[:], bn2sb[:, i], gate[:, i:i + 1],
                        xpad[:, i, 1:1 + H, 1:1 + W],
                        op0=ALU.mult, op1=ALU.add)
                    nc.scalar.activation(outsb[:, i], t2[:], AF.Relu)
                    nc.sync.dma_start(outa[i], outsb[:, i])

            enc2_img(0)
            conv2_img(0)
            enc2_img(1)
            conv2_img(1)
            tail_img(0)
            tail_img(1)

    nc.compile()
    return nc


_NC_CACHE = None


def _get_nc():
    global _NC_CACHE
    if _NC_CACHE is None:
        _prep_consts()
        _NC_CACHE = _build_nc()
    return _NC_CACHE


def _snap_fp8_up(x):
    """Smallest fp8e4m3-exact value >= x (normal range)."""
    cand = np.float32(x)
    f8 = np.float32(ml_dtypes.float8_e4m3(cand))
    while f8 < x:
        cand = np.nextafter(np.float32(cand * 1.01), np.float32(np.inf))
        f8 = np.float32(ml_dtypes.float8_e4m3(cand))
    return float(f8)


_G = {}


def _prep_consts():
    """Grid geometry is input-independent only given |w| bounds; computed in
    _host_prep and stashed for _build_nc (thresholds used as immediates)."""
    _build_nc.th1_dve = _G["th1_dve"]


def _host_prep(inputs):
    f, fd = np.float32, np.float64
    w1 = np.asarray(inputs["w1"], fd).reshape(C, C, KK)
    w2 = np.asarray(inputs["w2"], fd).reshape(C, C, KK)

    d1 = _snap_fp8_up(2.0 * np.abs(w1).max() * 1.0001 / T1)
    c1 = d1 * T1 / 2.0
    th1 = -c1 + (np.arange(T1) + 0.5) * d1
    d2 = _snap_fp8_up(np.abs(w2).max() * 1.0001 / T2P)
    c2 = d2 * T2P
    th2p = (np.arange(T2P) + 0.5) * d2

    _G["th1_dve"] = (float(th1[1]), float(th1[2]))

    def build_A(w, convs, ths, delta):
        A = np.empty((C, KK, len(convs), C), fd)      # [ci, o, p, co]
        mm0 = np.zeros(C, fd)
        for p, kind in enumerate(convs):
            Ap = _plane_w(kind, w, ths[p], delta)     # [co, ci, o]
            A[:, :, p, :] = Ap.transpose(1, 2, 0)
            mm0 += Ap.sum(axis=(1, 2)) * _enc0(kind, ths[p])
        return A, mm0

    A1, mm0_1 = build_A(w1, CONV1, th1, d1)
    A2, mm0_2 = build_A(w2, CONV2, th2p, d2)
    C1 = np.abs(w1).sum(axis=(1, 2)) - mm0_1
    C2 = np.abs(w2).sum(axis=(1, 2)) - mm0_2

    def pack_pairs(A):
        # [ci, o, p, co] -> [ci, o, j, k, co] fp8 bytes
        Ar = A.reshape(C, KK, 2, 2, C)
        A8v = Ar.astype(ml_dtypes.float8_e4m3)
        assert np.array_equal(A8v.astype(fd), Ar), "A not fp8-exact"
        return np.ascontiguousarray(A8v).view(np.uint8).reshape(C, -1)

    def bn_fold(g_, b_, m_, v_):
        g_, b_, m_, v_ = (np.asarray(t, fd) for t in (g_, b_, m_, v_))
        a = g_ / np.sqrt(v_ + EPS)
        return a, b_ - m_ * a

    a1, b1 = bn_fold(inputs["bn1_gamma"], inputs["bn1_beta"],
                     inputs["bn1_mean"], inputs["bn1_var"])
    a2, b2 = bn_fold(inputs["bn2_gamma"], inputs["bn2_beta"],
                     inputs["bn2_mean"], inputs["bn2_var"])
    alpha1, beta1f = -a1, b1 - a1 * C1
    alpha2, beta2f = -a2, b2 - a2 * C2

    bnc = np.zeros((C, 12), fd)
    bnc[:, 0] = alpha1
    bnc[:, 1] = alpha2
    bnc[:, 2] = beta2f
    bnc[:, 3] = -th1[0]
    bnc[:, 4] = -th1[3]
    bnc[:, 5] = -c1
    bnc[:, 6] = beta1f - th2p[0]
    bnc[:, 7] = beta1f - th2p[3]
    bnc[:, 8] = beta1f - c2
    bnc[:, 9] = th2p[1] - beta1f
    bnc[:, 10] = th2p[2] - beta1f

    fc1T = np.ascontiguousarray(inputs["fc1_w"].astype(f).T / np.float32(POS))
    fc1b = np.ascontiguousarray(inputs["fc1_b"].astype(f).reshape(8, 1))
    fc2T = np.ascontiguousarray(inputs["fc2_w"].astype(f).T)
    fc2b = np.ascontiguousarray(inputs["fc2_b"].astype(f).reshape(C, 1))

    return dict(
        A1=pack_pairs(A1), A2=pack_pairs(A2),
        bnc=np.ascontiguousarray(bnc, dtype=f),
        fc1T=fc1T, fc1b=fc1b, fc2T=fc2T, fc2b=fc2b)


def run(inputs, trace=False, tmpdir=None):
    shared = _host_prep(inputs)
    nc = _get_nc()
    x = np.ascontiguousarray(inputs["x"], dtype=np.float32)
    in_maps = []
    for i in range(N_CORES):
        m = dict(shared)
        m["x"] = np.ascontiguousarray(x[i * NPC:(i + 1) * NPC])
        in_maps.append(m)
    res = run_bass_kernel_spmd(nc, in_maps, core_ids=list(range(N_CORES)),
                               trace=trace, tmpdir=tmpdir)
    out = np.concatenate([res.results[i]["out"] for i in range(N_CORES)], 0)
    return out, res


def kernel(**inputs) -> np.ndarray:
    out, _ = run(inputs)
    return out


# revision 4
# speedup vs baseline: 2.6696x; 1.2901x over previous
"""AdderNet BasicBlock v3: pipelined threshold-matmul kernel. 8 TRN2 cores.

v2 -> v3 (trace-driven):
  - x/bnc DMAs issued first; weight DMAs follow (x was landing at 21.5us)
  - dummy Sigmoid activation up front: loads the one ACT table that serves
    Sign/Relu/Abs/Identity/Sigmoid during the DMA window (was 1.28us in tail)
  - enc2 restructured: per row-part ACT copies PSUM1 -> SBUF (fused BN1
    affine, so planes become plain threshold ops), then threshold encodes
    run from SBUF on ACT+DVE in parallel (PSUM banks allow only one reader)
    and overlap the remaining conv matmuls
  - SE/tail for image 0 interleaved between conv2-img1 chunks so PE never
    waits; BN2 done in row-parts with accum_out partial sums
  - warmup matmul count tuned; all engines stay busy so the HAM clock gate
    stays at 2.4 GHz through the conv phases
"""

import numpy as np
import ml_dtypes

import concourse.bacc as bacc
import concourse.bass as bass
import concourse.mybir as mybir
import concourse.tile as tile
from concourse.bass_utils import run_bass_kernel_spmd

F32 = mybir.dt.float32
F16 = mybir.dt.float16
F8 = mybir.dt.float8e4
U8 = mybir.dt.uint8
AF = mybir.ActivationFunctionType
ALU = mybir.AluOpType
PM = mybir.MatmulPerfMode

N_CORES = 8
N, C, H, W = 16, 128, 32, 32
NPC = N // N_CORES
HP, WP = H + 2, W + 2
PPI = HP * WP
POS = H * W
WIN = (H - 1) * WP + W           # 1086
KK = 9
EPS = 1e-5

T1 = 4
T2P = 4
OFFS = [dh * WP + dw for dh in range(3) for dw in range(3)]
CHUNKS = [(0, 512), (512, 512), (1024, WIN - 1024)]
ROWPARTS = [(0, 15), (15, 30), (30, 32)]   # rows done after chunk 0/1/2
N_WARM = 5

CONV1 = ("sign", "gt", "gt", "sign")
CONV2 = ("sign", "gt", "gt", "sign")


def _enc0(kind, th):
    return -np.sign(th) if kind == "sign" else float(0.0 > th)


def _plane_w(kind, w, th, delta):
    if kind == "sign":
        return -(delta / 2.0) * (2.0 * (w > th) - 1.0)
    return delta * (1.0 - 2.0 * (w > th))


def _build_nc():
    nc = bacc.Bacc("TRN2", target_bir_lowering=False, debug=False,
                   num_devices=N_CORES)

    x_d = nc.dram_tensor("x", [NPC, C, H, W], F32, kind="ExternalInput")
    A1_d = nc.dram_tensor("A1", [C, KK * T1 * C], U8, kind="ExternalInput")
    A2_d = nc.dram_tensor("A2", [C, KK * T2P * C], U8, kind="ExternalInput")
    bnc_d = nc.dram_tensor("bnc", [C, 16], F32, kind="ExternalInput")
    fc1T_d = nc.dram_tensor("fc1T", [C, 8], F32, kind="ExternalInput")
    fc1b_d = nc.dram_tensor("fc1b", [8, 1], F32, kind="ExternalInput")
    fc2T_d = nc.dram_tensor("fc2T", [8, C], F32, kind="ExternalInput")
    fc2b_d = nc.dram_tensor("fc2b", [C, 1], F32, kind="ExternalInput")
    out_d = nc.dram_tensor("out", [NPC, C, H, W], F32, kind="ExternalOutput")

    xa, outa = x_d.ap(), out_d.ap()
    th1_dve = _build_nc.th1_dve
    th2_dve = _build_nc.th2_dve      # (th2p[1], th2p[2]) immediates

    with tile.TileContext(nc) as tc:
        with (
            tc.tile_pool(name="const", bufs=1) as cpool,
            tc.tile_pool(name="pad", bufs=1) as padpool,
            tc.tile_pool(name="planes", bufs=1) as plpool,
            tc.tile_pool(name="misc", bufs=1) as mpool,
            tc.tile_pool(name="psum", bufs=2, space=bass.MemorySpace.PSUM) as pp,
            tc.tile_pool(name="psum_se", bufs=2, space=bass.MemorySpace.PSUM) as pps,
        ):
            ones = cpool.tile([C, C], F16, tag="ones")
            nc.vector.memset(ones[:], 1.0)
            scr = cpool.tile([C, 512], F16, tag="scr")
            nc.vector.memset(scr[:], 0.0)
            # ACT table preload: sigmoid's set also holds sign/relu/abs/identity
            tldt = cpool.tile([C, 1], F32, tag="tldt")
            nc.scalar.activation(tldt[:], scr[:, 0:1], AF.Sigmoid)

            # ---- DMAs: x + bnc first (enc1 critical path), weights after ----
            xpad = padpool.tile([C, NPC, HP, WP], F32, tag="xpad")
            nc.vector.memset(xpad[:, :, 0, :], 0.0)
            nc.vector.memset(xpad[:, :, HP - 1, :], 0.0)
            nc.vector.memset(xpad[:, :, 1:1 + H, 0], 0.0)
            nc.vector.memset(xpad[:, :, 1:1 + H, WP - 1], 0.0)
            for i in range(NPC):
                nc.sync.dma_start(xpad[:, i, 1:1 + H, 1:1 + W], xa[i])
            bnc = cpool.tile([C, 16], F32, tag="bnc")
            nc.sync.dma_start(bnc[:], bnc_d.ap())
            A1 = cpool.tile([C, KK, 2, 2, C], U8, tag="A1")
            nc.sync.dma_start(A1[:].rearrange("p a b c d -> p (a b c d)"), A1_d.ap())
            A2 = cpool.tile([C, KK, 2, 2, C], U8, tag="A2")
            nc.sync.dma_start(A2[:].rearrange("p a b c d -> p (a b c d)"), A2_d.ap())
            fc1T = cpool.tile([C, 8], F32, tag="fc1T")
            nc.sync.dma_start(fc1T[:], fc1T_d.ap())
            fc1b = cpool.tile([8, 1], F32, tag="fc1b")
            nc.sync.dma_start(fc1b[:], fc1b_d.ap())
            fc2T = cpool.tile([8, C], F32, tag="fc2T")
            nc.sync.dma_start(fc2T[:], fc2T_d.ap())
            fc2b = cpool.tile([C, 1], F32, tag="fc2b")
            nc.sync.dma_start(fc2b[:], fc2b_d.ap())

            # warm-up matmuls (PE FIFO head; HAM ramp)
            wps = pps.tile([C, 512], F32, tag="se")
            for _ in range(N_WARM):
                nc.tensor.matmul(wps[:], ones[:], scr[:], start=True, stop=True)

            def A8(t, o, j):
                return t[:, o, j].bitcast(F8)

            xpf = xpad[:].rearrange("p i h w -> p (i h w)")

            # ---- layer-1 encodings ----
            E1 = plpool.tile([C, 2, 2, NPC * PPI], F8, tag="E1")
            nc.vector.tensor_scalar(E1[:, 0, 1], xpf, th1_dve[0], None,
                                    op0=ALU.is_gt)
            nc.scalar.activation(E1[:, 0, 0], xpf, AF.Sign, bias=bnc[:, 3:4])
            nc.vector.tensor_scalar(E1[:, 1, 0], xpf, th1_dve[1], None,
                                    op0=ALU.is_gt)
            nc.scalar.activation(E1[:, 1, 1], xpf, AF.Sign, bias=bnc[:, 4:5])
            ab = padpool.tile([C, NPC, HP, WP], F16, tag="ab")
            nc.scalar.activation(ab[:].rearrange("p i h w -> p (i h w)"),
                                 xpf, AF.Abs)
            v1t = padpool.tile([C, NPC, HP, WP], F16, tag="v1t")
            nc.scalar.activation(v1t[:].rearrange("p i h w -> p (i h w)"),
                                 ab[:].rearrange("p i h w -> p (i h w)"),
                                 AF.Relu, bias=bnc[:, 5:6])
            rs1 = padpool.tile([C, NPC, HP, WP], F16, tag="rs1")
            nc.vector.tensor_tensor(rs1[:, :, :, 0:W], v1t[:, :, :, 0:W],
                                    v1t[:, :, :, 1:1 + W], op=ALU.add)
            nc.vector.tensor_tensor(rs1[:, :, :, 0:W], rs1[:, :, :, 0:W],
                                    v1t[:, :, :, 2:2 + W], op=ALU.add)
            vs1 = padpool.tile([C, NPC, HP, WP], F16, tag="vs1")
            nc.vector.tensor_tensor(vs1[:, :, 0:H, :], rs1[:, :, 0:H, :],
                                    rs1[:, :, 1:1 + H, :], op=ALU.add)
            nc.vector.tensor_tensor(vs1[:, :, 0:H, :], vs1[:, :, 0:H, :],
                                    rs1[:, :, 2:2 + H, :], op=ALU.add)
            vs1f = vs1[:].rearrange("p i h w -> p (i h w)")

            # ---- layer-2 plane pad strips (GPSIMD) ----
            E2 = plpool.tile([C, 2, 2, NPC, HP, WP], F8, tag="E2")
            PADV = {(0, 0): -1.0, (0, 1): 0.0, (1, 0): 0.0, (1, 1): -1.0}
            for (j, k), pv in PADV.items():
                pl = E2[:, j, k]
                nc.gpsimd.memset(pl[:, :, 0, :], pv)
                nc.gpsimd.memset(pl[:, :, HP - 1, :], pv)
                nc.gpsimd.memset(pl[:, :, 1:1 + H, 0], pv)
                nc.gpsimd.memset(pl[:, :, 1:1 + H, WP - 1], pv)
            v2t = padpool.tile([C, NPC, HP, WP], F16, tag="v2t")
            nc.gpsimd.memset(v2t[:, :, 0, :], 0.0)
            nc.gpsimd.memset(v2t[:, :, HP - 1, :], 0.0)
            nc.gpsimd.memset(v2t[:, :, 1:1 + H, 0], 0.0)
            nc.gpsimd.memset(v2t[:, :, 1:1 + H, WP - 1], 0.0)

            # layer-1 conv matmuls for one image
            psum1 = [pp.tile([C, 3 * 512], F32, tag="conv", name=f"psum1_{i_}")
                     for i_ in range(NPC)]

            def conv_img(Eg, vsf, At, ps, i, scope):
                with nc.named_scope(scope):
                    for base, Lc in CHUNKS:
                        first = True
                        for j in range(2):
                            for o in range(KK):
                                w0 = i * PPI + OFFS[o] + base
                                nc.tensor.matmul(
                                    ps[:, base:base + Lc], A8(At, o, j),
                                    Eg[:, j, :, w0:w0 + Lc],
                                    start=first, stop=False,
                                    perf_mode=PM.DoubleRow)
                                first = False
                        nc.tensor.matmul(
                            ps[:, base:base + Lc], ones[:],
                            vsf[:, i * PPI + base:i * PPI + base + Lc],
                            start=False, stop=True)

            e1v = E1[:]
            conv_img(e1v, vs1f, A1, psum1[0], 0, "conv1")
            conv_img(e1v, vs1f, A1, psum1[1], 1, "conv1")

            # ---- enc2: PSUM1 -> SBUF (BN1 fused), then threshold encodes ----
            s2buf = mpool.tile([C, NPC, H, W], F32, tag="s2buf")
            rs2 = padpool.tile([C, NPC, HP, WP], F16, tag="rs2")
            vs2 = padpool.tile([C, NPC, HP, WP], F16, tag="vs2")
            vs2f = vs2[:].rearrange("p i h w -> p (i h w)")
            e2v = E2[:].rearrange("p j k i h w -> p j k (i h w)")
            psum2 = []

            def enc2_part(i, r0, r1):
                pv = psum1[i][:, 0:H * WP].rearrange(
                    "p (h w) -> p h w", h=H, w=WP)[:, r0:r1, 0:W]
                # BN1-affine into SBUF: y = alpha1*S + beta1f (pre-relu out1)
                nc.scalar.activation(s2buf[:, i, r0:r1, :], pv, AF.Identity,
                                     bias=bnc[:, 6:7], scale=bnc[:, 0:1])
                sb = s2buf[:, i, r0:r1, :]
                nc.scalar.activation(E2[:, 0, 0, i, 1 + r0:1 + r1, 1:1 + W],
                                     sb, AF.Sign, bias=bnc[:, 7:8])
                nc.vector.tensor_scalar(E2[:, 0, 1, i, 1 + r0:1 + r1, 1:1 + W],
                                        sb, th2_dve[0], None, op0=ALU.is_gt)
                nc.vector.tensor_scalar(E2[:, 1, 0, i, 1 + r0:1 + r1, 1:1 + W],
                                        sb, th2_dve[1], None, op0=ALU.is_gt)
                nc.scalar.activation(E2[:, 1, 1, i, 1 + r0:1 + r1, 1:1 + W],
                                     sb, AF.Sign, bias=bnc[:, 8:9])
                nc.scalar.activation(v2t[:, i, 1 + r0:1 + r1, 1:1 + W],
                                     sb, AF.Relu, bias=bnc[:, 9:10])

            def box2_img(i):
                nc.vector.tensor_tensor(rs2[:, i, :, 0:W], v2t[:, i, :, 0:W],
                                        v2t[:, i, :, 1:1 + W], op=ALU.add)
                nc.vector.tensor_tensor(rs2[:, i, :, 0:W], rs2[:, i, :, 0:W],
                                        v2t[:, i, :, 2:2 + W], op=ALU.add)
                nc.vector.tensor_tensor(vs2[:, i, 0:H, :], rs2[:, i, 0:H, :],
                                        rs2[:, i, 1:1 + H, :], op=ALU.add)
                nc.vector.tensor_tensor(vs2[:, i, 0:H, :], vs2[:, i, 0:H, :],
                                        rs2[:, i, 2:2 + H, :], op=ALU.add)

            def enc2_img(i):
                with nc.named_scope("enc2"):
                    for r0, r1 in ROWPARTS:
                        enc2_part(i, r0, r1)
                    box2_img(i)

            def conv2_img(i, se_hook=None):
                ps = pp.tile([C, 3 * 512], F32, tag="conv", name=f"psum2_{i}")
                psum2.append(ps)
                with nc.named_scope("conv2"):
                    for ci, (base, Lc) in enumerate(CHUNKS):
                        if se_hook is not None and ci == 2:
                            se_hook()
                        first = True
                        for j in range(2):
                            for o in range(KK):
                                w0 = i * PPI + OFFS[o] + base
                                nc.tensor.matmul(
                                    ps[:, base:base + Lc], A8(A2, o, j),
                                    e2v[:, j, :, w0:w0 + Lc],
                                    start=first, stop=False,
                                    perf_mode=PM.DoubleRow)
                                first = False
                        nc.tensor.matmul(
                            ps[:, base:base + Lc], ones[:],
                            vs2f[:, i * PPI + base:i * PPI + base + Lc],
                            start=False, stop=True)

            bn2sb = mpool.tile([C, NPC, H, W], F32, tag="bn2sb")
            pooled3 = mpool.tile([C, NPC, 3], F32, tag="pooled3")
            pooled = mpool.tile([C, NPC], F32, tag="pooled")
            s2g = mpool.tile([8, NPC], F32, tag="s2g")
            gate = mpool.tile([C, NPC], F32, tag="gate")
            outsb = mpool.tile([C, NPC, H, W], F32, tag="outsb")

            def bn2_img(i):
                with nc.named_scope("tail"):
                    for pi, (r0, r1) in enumerate(ROWPARTS):
                        pv = psum2[i][:, 0:H * WP].rearrange(
                            "p (h w) -> p h w", h=H, w=WP)[:, r0:r1, 0:W]
                        nc.scalar.activation(
                            bn2sb[:, i, r0:r1, :], pv, AF.Identity,
                            bias=bnc[:, 11:12], scale=bnc[:, 10:11],
                            accum_out=pooled3[:, i, pi:pi + 1])
                    nc.vector.reduce_sum(pooled[:, i:i + 1], pooled3[:, i],
                                         axis=mybir.AxisListType.X)

            def se_mms(i):
                ps1 = pps.tile([8, 1], F32, tag="se", name=f"ps1_{i}")
                nc.tensor.matmul(ps1[:], fc1T[:], pooled[:, i:i + 1],
                                 start=True, stop=True)
                nc.scalar.activation(s2g[:, i:i + 1], ps1[:], AF.Relu,
                                     bias=fc1b[:, 0:1])
                ps2_ = pps.tile([C, 1], F32, tag="se", name=f"ps2_{i}")
                nc.tensor.matmul(ps2_[:], fc2T[:], s2g[:, i:i + 1],
                                 start=True, stop=True)
                nc.scalar.activation(gate[:, i:i + 1], ps2_[:], AF.Sigmoid,
                                     bias=fc2b[:, 0:1])

            def resid_img(i):
                with nc.named_scope("tail"):
                    t2 = mpool.tile([C, H, W], F32, tag="t2", name=f"t2_{i}")
                    nc.vector.scalar_tensor_tensor(
                        t2[:], bn2sb[:, i], gate[:, i:i + 1],
                        xpad[:, i, 1:1 + H, 1:1 + W],
                        op0=ALU.mult, op1=ALU.add)
                    nc.scalar.activation(outsb[:, i], t2[:], AF.Relu)
                    nc.sync.dma_start(outa[i], outsb[:, i])

            enc2_img(0)
            conv2_img(0)
            enc2_img(1)
            bn2_img(0)

            def se0_hook():
                se_mms(0)

            conv2_img(1, se_hook=se0_hook)
            resid_img(0)
            bn2_img(1)
            se_mms(1)
            resid_img(1)

    nc.compile()
    return nc


_NC_CACHE = None


def _get_nc():
    global _NC_CACHE
    if _NC_CACHE is None:
        _build_nc.th1_dve = _G["th1_dve"]
        _build_nc.th2_dve = _G["th2_dve"]
        _NC_CACHE = _build_nc()
    return _NC_CACHE


def _snap_fp8_up(x):
    cand = np.float32(x)
    f8 = np.float32(ml_dtypes.float8_e4m3(cand))
    while f8 < x:
        cand = np.nextafter(np.float32(cand * 1.01), np.float32(np.inf))
        f8 = np.float32(ml_dtypes.float8_e4m3(cand))
    return float(f8)


_G = {}


def _host_prep(inputs):
    f, fd = np.float32, np.float64
    w1 = np.asarray(inputs["w1"], fd).reshape(C, C, KK)
    w2 = np.asarray(inputs["w2"], fd).reshape(C, C, KK)

    d1 = _snap_fp8_up(2.0 * np.abs(w1).max() * 1.0001 / T1)
    c1 = d1 * T1 / 2.0
    th1 = -c1 + (np.arange(T1) + 0.5) * d1
    d2 = _snap_fp8_up(np.abs(w2).max() * 1.0001 / T2P)
    c2 = d2 * T2P
    th2p = (np.arange(T2P) + 0.5) * d2

    _G["th1_dve"] = (float(th1[1]), float(th1[2]))
    _G["th2_dve"] = (float(th2p[1]), float(th2p[2]))

    def build_A(w, convs, ths, delta):
        A = np.empty((C, KK, len(convs), C), fd)
        mm0 = np.zeros(C, fd)
        for p, kind in enumerate(convs):
            Ap = _plane_w(kind, w, ths[p], delta)
            A[:, :, p, :] = Ap.transpose(1, 2, 0)
            mm0 += Ap.sum(axis=(1, 2)) * _enc0(kind, ths[p])
        return A, mm0

    A1, mm0_1 = build_A(w1, CONV1, th1, d1)
    A2, mm0_2 = build_A(w2, CONV2, th2p, d2)
    C1 = np.abs(w1).sum(axis=(1, 2)) - mm0_1
    C2 = np.abs(w2).sum(axis=(1, 2)) - mm0_2

    def pack_pairs(A):
        Ar = A.reshape(C, KK, 2, 2, C)
        A8v = Ar.astype(ml_dtypes.float8_e4m3)
        assert np.array_equal(A8v.astype(fd), Ar), "A not fp8-exact"
        return np.ascontiguousarray(A8v).view(np.uint8).reshape(C, -1)

    def bn_fold(g_, b_, m_, v_):
        g_, b_, m_, v_ = (np.asarray(t, fd) for t in (g_, b_, m_, v_))
        a = g_ / np.sqrt(v_ + EPS)
        return a, b_ - m_ * a

    a1, b1 = bn_fold(inputs["bn1_gamma"], inputs["bn1_beta"],
                     inputs["bn1_mean"], inputs["bn1_var"])
    a2, b2 = bn_fold(inputs["bn2_gamma"], inputs["bn2_beta"],
                     inputs["bn2_mean"], inputs["bn2_var"])
    alpha1, beta1f = -a1, b1 - a1 * C1
    alpha2, beta2f = -a2, b2 - a2 * C2

    bnc = np.zeros((C, 16), fd)
    bnc[:, 0] = alpha1              # enc2 copy scale
    bnc[:, 3] = -th1[0]             # enc1 ACT plane biases
    bnc[:, 4] = -th1[3]
    bnc[:, 5] = -c1                 # v1 bias
    bnc[:, 6] = beta1f              # enc2 copy bias
    bnc[:, 7] = -th2p[0]            # enc2 plane biases (from s2buf)
    bnc[:, 8] = -th2p[3]
    bnc[:, 9] = -c2                 # v2 bias
    bnc[:, 10] = alpha2             # bn2 scale
    bnc[:, 11] = beta2f             # bn2 bias

    fc1T = np.ascontiguousarray(inputs["fc1_w"].astype(f).T / np.float32(POS))
    fc1b = np.ascontiguousarray(inputs["fc1_b"].astype(f).reshape(8, 1))
    fc2T = np.ascontiguousarray(inputs["fc2_w"].astype(f).T)
    fc2b = np.ascontiguousarray(inputs["fc2_b"].astype(f).reshape(C, 1))

    return dict(
        A1=pack_pairs(A1), A2=pack_pairs(A2),
        bnc=np.ascontiguousarray(bnc, dtype=f),
        fc1T=fc1T, fc1b=fc1b, fc2T=fc2T, fc2b=fc2b)


def run(inputs, trace=False, tmpdir=None):
    shared = _host_prep(inputs)
    nc = _get_nc()
    x = np.ascontiguousarray(inputs["x"], dtype=np.float32)
    in_maps = []
    for i in range(N_CORES):
        m = dict(shared)
        m["x"] = np.ascontiguousarray(x[i * NPC:(i + 1) * NPC])
        in_maps.append(m)
    res = run_bass_kernel_spmd(nc, in_maps, core_ids=list(range(N_CORES)),
                               trace=trace, tmpdir=tmpdir)
    out = np.concatenate([res.results[i]["out"] for i in range(N_CORES)], 0)
    return out, res


def kernel(**inputs) -> np.ndarray:
    out, _ = run(inputs)
    return out
